# revision 41
# baseline (speedup 1.0000x reference)
"""Trainium2 Bass kernel for nn_BasicBlock_37228776522121 (binary-conv BasicBlock).

Computes: y = conv3x3(binarize(x), sign(W)*alpha) -> BN(inference) -> + x
  where binarize(x) = sign(x) in {-1,+1} (sign(0)=+1),
        alpha[o] = mean(|W[o]|), folded into the BN scale.

Strategy: data-parallel over batch N across 8 NeuronCores (8 images/core).
Per core: conv as 9 shifted matmuls, contraction C=256 done in ONE PE pass
via fp8 DoubleRow (2 K-tiles of 128), sign activations in fp8e4 (exact for
+-1), f32 PSUM accumulation (exact integer sums). BN affine + residual are
fused into a single DVE scalar_tensor_tensor evacuation per PSUM chunk
(bias pre-added to the residual x off the critical path).

Layout trick: activations stored sign-binarized in a width-padded layout
[128, 2, 58*58] so every conv tap (ky,kx) is a contiguous slice at offset
ky*58+kx. Output positions p = h*58+w include 2 garbage columns per row
(w=56,57) skipped when evacuating PSUM (strided 8x56 reads from each
464-wide chunk = 8 padded rows).
"""

import os
import sys

import numpy as np

for _p in ("/opt/trn_rl_repo",):
    if _p not in sys.path and os.path.isdir(_p):
        sys.path.insert(0, _p)

import concourse.bacc as bacc
import concourse.bass as bass
import concourse.tile as tile
from concourse import mybir

# ---- problem constants (hardcoded per contest rules) ----
N_FULL = 64
C = 256  # = Cout
H = 56
W_DIM = 56
KH = KW = 3
BN_EPS = 1e-5
N_CORES = 8
N_IMG = N_FULL // N_CORES  # images per core

P = 128  # partitions
NT = C // P  # channel tiles (2)
HW = H * W_DIM  # 3136
WP = W_DIM + 2  # 58 padded width
HP = H + 2  # 58 padded height
NPAD = HP * WP  # 3364
ROWS_PER_CHUNK = 8
CH = ROWS_PER_CHUNK * WP  # 464 padded positions / chunk
CHV = ROWS_PER_CHUNK * W_DIM  # 448 valid positions / chunk
NCHUNK = H // ROWS_PER_CHUNK  # 7
NPOS = (H - 1) * WP + W_DIM  # 3246 last valid padded index + 1

F32 = mybir.dt.float32
BF16 = mybir.dt.bfloat16
FP8 = mybir.dt.float8e4
APAD = 3376  # NPAD rounded up to 16 for DoubleRow k-tile stride alignment

# set by test.py to collect profile info
TRACE = False
LAST_RESULTS = None


def build_nc(n_img: int = N_IMG):
    nc = bacc.Bacc("TRN2", target_bir_lowering=False)

    x_d = nc.declare_dram_parameter("x", [n_img, C, HW], F32, isOutput=False)
    # weights pre-permuted on host to [128ci, ky*3+kx, ci_tile, 256co] (layout
    # only) so the stage DMA is fully contiguous per partition
    wt_d = nc.declare_dram_parameter("w_t", [P, KH * KW * NT * C], F32, isOutput=False)
    # native-layout weights for alpha: [co_tile, 128co, C*9]
    wn_d = nc.declare_dram_parameter("w_n", [NT, P, C * KH * KW], F32, isOutput=False)
    # gamma/beta/mean/var packed host-side partition-major [128p, 4param, 2ct]
    # so all BN params arrive in ONE contiguous-per-partition DMA (a scattered
    # 4B-element gather here measurably stalls the ACT queue ~20us at startup)
    bn_d = nc.declare_dram_parameter("bn", [P, 4, NT], F32, isOutput=False)
    out_d = nc.declare_dram_parameter("out", [n_img, C, HW], F32, isOutput=True)

    with tile.TileContext(nc) as tc:
        with (
            tc.tile_pool(name="wpool", bufs=1) as wpool,
            tc.tile_pool(name="ppool", bufs=1) as ppool,
            tc.tile_pool(name="xpool", bufs=3) as xpool,
            tc.tile_pool(name="apool", bufs=3) as apool,
            tc.tile_pool(name="pspool", bufs=8, space="PSUM") as pspool,
        ):
            # ---- consts + BN params (tiny DMAs; bias needs no weights) ----
            tiny_t = ppool.tile([P, 1], F32, name="tiny_t")
            nc.vector.memset(tiny_t, 1e-30)
            eps_t = ppool.tile([P, 1], F32, name="eps_t")
            nc.vector.memset(eps_t, BN_EPS)
            # warm the ACT function tables now so their table-data DMAs queue
            # ahead of the bulk x/W transfers (otherwise the first real Sign
            # stalls ~15us mid-startup waiting for its table)
            warm_t = ppool.tile([P, 1], F32, name="warm_t")
            nc.scalar.activation(
                out=warm_t, in_=tiny_t, func=mybir.ActivationFunctionType.Sign, bias=tiny_t
            )
            nc.scalar.activation(
                out=warm_t, in_=warm_t, func=mybir.ActivationFunctionType.Sqrt, bias=eps_t
            )

            # one contiguous DMA for all BN params; computes are emitted LATER
            # (after the image-0 signs) so they can't convoy the ACT queue.
            # Tiles are allocated now so emit_inputs can reference bias_t.
            bn_t = ppool.tile([P, 4, NT], F32, name="bn_t")
            nc.sync.dma_start(out=bn_t, in_=bn_d[:])
            inv_t = [ppool.tile([P, 1], F32, name=f"inv{c}", tag=f"inv{c}") for c in range(NT)]
            bias_t = [ppool.tile([P, 1], F32, name=f"bb{c}", tag=f"bb{c}") for c in range(NT)]
            scale_t = [ppool.tile([P, 1], F32, name=f"s{c}", tag=f"s{c}") for c in range(NT)]

            # ---------------- per-image input pipeline ----------------
            prev_x_dma = [None]  # serialize x transfers in image order so
            # image n's data never steals fabric bandwidth from image n-1

            def emit_inputs(n, do_bias=True):
                """DMA x, zero pad borders, sign -> fp8 a-tile, pre-bias x."""
                xlin = []
                # both ci-halves in one fp8 tile: [p, 2, APAD] (DoubleRow rhs)
                at = apool.tile([P, NT * APAD], FP8, name=f"a_{n}", tag="a")
                a2 = at.rearrange("p (u q) -> p u q", u=NT)
                HH = H // 2  # row-halves: lets early-chunk matmuls start
                first_dma, last_dma = None, None
                for u in range(NT):
                    xt = xpool.tile([P, HW], F32, name=f"x_{n}_{u}", tag=f"x{u}")
                    a3 = a2[:, u, 0:NPAD].rearrange("p (h w) -> p h w", w=WP)
                    # zero borders (rows 0,57 and cols 0,57)
                    nc.gpsimd.memset(a3[:, 0, :], 0.0)
                    nc.gpsimd.memset(a3[:, HP - 1, :], 0.0)
                    nc.gpsimd.memset(a3[:, :, 0], 0.0)
                    nc.gpsimd.memset(a3[:, :, WP - 1], 0.0)
                    for h in range(2):
                        rs = slice(h * HH * W_DIM, (h + 1) * HH * W_DIM)
                        # u0 rides the sync ring, u1 the gpsimd ring
                        eng = nc.sync if u == 0 else nc.gpsimd
                        dma = eng.dma_start(
                            out=xt[:, rs], in_=x_d[n, u * P : (u + 1) * P, rs]
                        )
                        first_dma = first_dma or dma
                        last_dma = dma
                        # sign(x) -> interior; sign(0)=+1 via tiny bias
                        nc.scalar.activation(
                            out=a3[:, 1 + h * HH : 1 + (h + 1) * HH, 1 : W_DIM + 1],
                            in_=xt[:, rs].rearrange("p (h w) -> p h w", w=W_DIM),
                            func=mybir.ActivationFunctionType.Sign,
                            bias=tiny_t,
                        )
                    # xb = x + bias, so PSUM evac is a single (S*scale)+xb op.
                    # Tile orders this in-place write after the sign read (WAR).
                    # Image 0's pre-bias is emitted AFTER the bias computation
                    # below (must not read bias_t before its writer exists).
                    if do_bias:
                        nc.vector.tensor_scalar_add(out=xt, in0=xt, scalar1=bias_t[u])
                    xlin.append(xt)
                if prev_x_dma[0] is not None:
                    tile.add_dep_helper(
                        first_dma.ins,
                        prev_x_dma[0].ins,
                        sync=True,
                        reason="image-order x DMA priority",
                    )
                prev_x_dma[0] = last_dma
                return xlin, a2

            pending = {0: emit_inputs(0, do_bias=False)}

            # ---- weight stage on the scalar HWDGE ring: runs in parallel
            # with image-0's x DMA (sync ring). Contiguous per-partition load.
            wstage = wpool.tile([P, KH * KW * NT * C], F32, name="wstage")
            nc.scalar.dma_start(out=wstage, in_=wt_d[:])
            # binarize weights on DVE (keeps the ACT queue free for the image
            # signs): (w >= 0) -> {0,1}, then *2-1 -> {-1,+1}; sign(0)=+1 exact
            wsgn = wpool.tile([P, KH * KW * NT * C], FP8, name="wsgn")
            nc.vector.tensor_scalar(
                out=wsgn, in0=wstage, scalar1=0.0, scalar2=None,
                op0=mybir.AluOpType.is_ge,
            )
            nc.vector.tensor_scalar(
                out=wsgn, in0=wsgn, scalar1=2.0, scalar2=-1.0,
                op0=mybir.AluOpType.mult, op1=mybir.AluOpType.add,
            )
            # [p, tap, ci_half, co] view for DoubleRow lhsT slices
            wsgn4 = wsgn.rearrange("p (t u co) -> p t u co", u=NT, co=C)

            # BN param math (ACT sqrt sits AFTER the image-0 signs + wsgn sign
            # in the ACT queue, so a slow bn DMA cannot delay the first matmul)
            for c in range(NT):
                g_t = bn_t[:, 0, c : c + 1]
                b_t = bn_t[:, 1, c : c + 1]
                m_t = bn_t[:, 2, c : c + 1]
                v_t = bn_t[:, 3, c : c + 1]
                iv, bb = inv_t[c], bias_t[c]
                # inv = gamma / sqrt(var + eps)
                nc.scalar.activation(
                    out=iv, in_=v_t, func=mybir.ActivationFunctionType.Sqrt, bias=eps_t
                )
                nc.vector.reciprocal(out=iv, in_=iv)
                nc.vector.tensor_mul(out=iv, in0=iv, in1=g_t)
                # bias = beta - mean * inv  (pre-added to residual x)
                nc.vector.tensor_mul(out=bb, in0=m_t, in1=iv)
                nc.vector.tensor_sub(out=bb, in0=b_t, in1=bb)

            # image 0's deferred pre-bias (bias_t now has a writer)
            for u in range(NT):
                xt0 = pending[0][0][u]
                nc.vector.tensor_scalar_add(out=xt0, in0=xt0, scalar1=bias_t[u])

            # alpha (mean |W| per co) -> scale = alpha * inv; scalar ring so it
            # queues behind wstage, not in front of x input DMAs
            for c in range(NT):
                wnat = wpool.tile([P, C * KH * KW], F32, name=f"wnat{c}", tag=f"wnat{c}")
                nc.scalar.dma_start(out=wnat, in_=wn_d[c])
                ar = ppool.tile([P, 1], F32, name=f"araw{c}", tag=f"araw{c}")
                nc.vector.reduce_sum(
                    out=ar, in_=wnat, axis=mybir.AxisListType.X, apply_absolute_value=True
                )
                st = scale_t[c]
                nc.vector.tensor_mul(out=st, in0=ar, in1=inv_t[c])
                nc.vector.tensor_scalar_mul(out=st, in0=st, scalar1=1.0 / (C * KH * KW))

            # ---------------- main loop over images ----------------
            PREFETCH = 2
            if n_img > 1:
                pending[1] = emit_inputs(1)
            for n in range(n_img):
                if n + PREFETCH < n_img:
                    pending[n + PREFETCH] = emit_inputs(n + PREFETCH)
                xlin, a2 = pending.pop(n)

                for c in range(NT):
                    for k in range(NCHUNK):
                        w_k = min(CH, NPOS - k * CH)  # 464, last 462
                        ps = pspool.tile([P, CH], F32, name=f"ps_{n}_{c}_{k}", tag="ps")
                        for t in range(KH * KW):
                            ky, kx = divmod(t, KW)
                            off = ky * WP + kx
                            nc.tensor.matmul(
                                ps[:, :w_k],
                                lhsT=wsgn4[:, t, :, c * P : (c + 1) * P],
                                rhs=a2[:, :, k * CH + off : k * CH + off + w_k],
                                start=(t == 0),
                                stop=(t == KH * KW - 1),
                                perf_mode=mybir.MatmulPerfMode.DoubleRow,
                            )
                        # evacuate in one DVE op: xb += S*scale (skip garbage cols)
                        ps3 = ps.rearrange("p (r w) -> p r w", w=WP)
                        xsl3 = xlin[c][:, k * CHV : (k + 1) * CHV].rearrange(
                            "p (r w) -> p r w", w=W_DIM
                        )
                        nc.vector.scalar_tensor_tensor(
                            out=xsl3,
                            in0=ps3[:, :, 0:W_DIM],
                            scalar=scale_t[c],
                            in1=xsl3,
                            op0=mybir.AluOpType.mult,
                            op1=mybir.AluOpType.add,
                        )
                    # outputs ride the scalar ring so they never convoy x
                    # inputs; two halves so the first can fire while the last
                    # chunks still compute
                    for os in (slice(0, 4 * CHV), slice(4 * CHV, HW)):
                        nc.scalar.dma_start(
                            out=out_d[n, c * P : (c + 1) * P, os], in_=xlin[c][:, os]
                        )
    return nc


def kernel(x, W, gamma, beta, running_mean, running_var):
    global LAST_RESULTS
    from concourse.bass_utils import run_bass_kernel_spmd

    assert x.shape == (N_FULL, C, H, W_DIM) and W.shape == (C, C, KH, KW)

    # host-side layout-only prep (no arithmetic): shard batch, permute weights
    xs = np.ascontiguousarray(x.reshape(N_CORES, N_IMG, C, HW))
    # [Cout, Cin, kh, kw] -> [kh,kw][ci_tile][128ci][256co] -> [p, (t u co)]
    w_t = np.ascontiguousarray(
        W.transpose(2, 3, 1, 0)
        .reshape(KH * KW, NT, P, C)
        .transpose(2, 0, 1, 3)
        .reshape(P, KH * KW * NT * C)
    )
    w_n = np.ascontiguousarray(W.reshape(NT, P, C * KH * KW))
    # [4, C] -> [128p, 4param, 2ct] partition-major
    bn = np.ascontiguousarray(
        np.stack([gamma, beta, running_mean, running_var])
        .astype(np.float32)
        .reshape(4, NT, P)
        .transpose(2, 0, 1)
    )

    in_maps = [
        {"x": xs[i], "w_t": w_t, "w_n": w_n, "bn": bn} for i in range(N_CORES)
    ]

    nc = build_nc(N_IMG)
    nc.finalize()
    res = run_bass_kernel_spmd(nc, in_maps, core_ids=list(range(N_CORES)), trace=TRACE)
    LAST_RESULTS = res
    out = np.concatenate([r["out"] for r in res.results], axis=0)
    return np.ascontiguousarray(out.reshape(N_FULL, C, H, W_DIM).astype(np.float32))


# revision 45
# speedup vs baseline: 1.0381x; 1.0381x over previous
"""Trainium2 Bass kernel for nn_BasicBlock_37228776522121 (binary-conv BasicBlock).

Computes: y = conv3x3(binarize(x), sign(W)*alpha) -> BN(inference) -> + x
  where binarize(x) = sign(x) in {-1,+1} (sign(0)=+1),
        alpha[o] = mean(|W[o]|), folded into the BN scale.

Strategy: data-parallel over batch N across 8 NeuronCores (8 images/core).
Per core: conv as 9 shifted matmuls, contraction C=256 done in ONE PE pass
via fp8 DoubleRow (2 K-tiles of 128), sign activations in fp8e4 (exact for
+-1), f32 PSUM accumulation (exact integer sums). BN affine + residual are
fused into a single DVE scalar_tensor_tensor evacuation per PSUM chunk
(bias pre-added to the residual x off the critical path).

Layout trick: activations stored sign-binarized in a width-padded layout
[128, 2, 58*58] so every conv tap (ky,kx) is a contiguous slice at offset
ky*58+kx. Output positions p = h*58+w include 2 garbage columns per row
(w=56,57) skipped when evacuating PSUM (strided 8x56 reads from each
464-wide chunk = 8 padded rows).
"""

import os
import sys

import numpy as np

for _p in ("/opt/trn_rl_repo",):
    if _p not in sys.path and os.path.isdir(_p):
        sys.path.insert(0, _p)

import concourse.bacc as bacc
import concourse.bass as bass
import concourse.tile as tile
from concourse import mybir

# ---- problem constants (hardcoded per contest rules) ----
N_FULL = 64
C = 256  # = Cout
H = 56
W_DIM = 56
KH = KW = 3
BN_EPS = 1e-5
N_CORES = 8
N_IMG = N_FULL // N_CORES  # images per core

P = 128  # partitions
NT = C // P  # channel tiles (2)
HW = H * W_DIM  # 3136
WP = W_DIM + 2  # 58 padded width
HP = H + 2  # 58 padded height
NPAD = HP * WP  # 3364
ROWS_PER_CHUNK = 8
CH = ROWS_PER_CHUNK * WP  # 464 padded positions / chunk
CHV = ROWS_PER_CHUNK * W_DIM  # 448 valid positions / chunk
NCHUNK = H // ROWS_PER_CHUNK  # 7
NPOS = (H - 1) * WP + W_DIM  # 3246 last valid padded index + 1

F32 = mybir.dt.float32
BF16 = mybir.dt.bfloat16
FP8 = mybir.dt.float8e4
APAD = 3376  # NPAD rounded up to 16 for DoubleRow k-tile stride alignment

# set by test.py to collect profile info
TRACE = False
LAST_RESULTS = None


def build_nc(n_img: int = N_IMG):
    nc = bacc.Bacc("TRN2", target_bir_lowering=False)

    x_d = nc.declare_dram_parameter("x", [n_img, C, HW], F32, isOutput=False)
    # weights pre-permuted on host to [128ci, ky*3+kx, ci_tile, 256co] (layout
    # only) so the stage DMA is fully contiguous per partition
    wt_d = nc.declare_dram_parameter("w_t", [P, KH * KW * NT * C], F32, isOutput=False)
    # native-layout weights for alpha: [co_tile, 128co, C*9]
    wn_d = nc.declare_dram_parameter("w_n", [NT, P, C * KH * KW], F32, isOutput=False)
    # gamma/beta/mean/var packed host-side partition-major [128p, 4param, 2ct]
    # so all BN params arrive in ONE contiguous-per-partition DMA (a scattered
    # 4B-element gather here measurably stalls the ACT queue ~20us at startup)
    bn_d = nc.declare_dram_parameter("bn", [P, 4, NT], F32, isOutput=False)
    out_d = nc.declare_dram_parameter("out", [n_img, C, HW], F32, isOutput=True)

    with tile.TileContext(nc) as tc:
        with (
            tc.tile_pool(name="wpool", bufs=1) as wpool,
            tc.tile_pool(name="ppool", bufs=1) as ppool,
            tc.tile_pool(name="xpool", bufs=3) as xpool,
            tc.tile_pool(name="apool", bufs=3) as apool,
            tc.tile_pool(name="pspool", bufs=8, space="PSUM") as pspool,
        ):
            # ---- consts + BN params (tiny DMAs; bias needs no weights) ----
            tiny_t = ppool.tile([P, 1], F32, name="tiny_t")
            nc.vector.memset(tiny_t, 1e-30)
            eps_t = ppool.tile([P, 1], F32, name="eps_t")
            nc.vector.memset(eps_t, BN_EPS)
            # warm the ACT function tables now so their table-data DMAs queue
            # ahead of the bulk x/W transfers (otherwise the first real Sign
            # stalls ~15us mid-startup waiting for its table)
            warm_t = ppool.tile([P, 1], F32, name="warm_t")
            nc.scalar.activation(
                out=warm_t, in_=tiny_t, func=mybir.ActivationFunctionType.Sign, bias=tiny_t
            )
            nc.scalar.activation(
                out=warm_t, in_=warm_t, func=mybir.ActivationFunctionType.Sqrt, bias=eps_t
            )

            # one contiguous DMA for all BN params; computes are emitted LATER
            # (after the image-0 signs) so they can't convoy the ACT queue.
            # Tiles are allocated now so emit_inputs can reference bias_t.
            bn_t = ppool.tile([P, 4, NT], F32, name="bn_t")
            nc.sync.dma_start(out=bn_t, in_=bn_d[:])
            inv_t = [ppool.tile([P, 1], F32, name=f"inv{c}", tag=f"inv{c}") for c in range(NT)]
            bias_t = [ppool.tile([P, 1], F32, name=f"bb{c}", tag=f"bb{c}") for c in range(NT)]
            scale_t = [ppool.tile([P, 1], F32, name=f"s{c}", tag=f"s{c}") for c in range(NT)]

            # ---- weight stage split across the scalar + gpsimd rings so the
            # two halves transfer concurrently (a single ~2.3MB DMA measures
            # only ~180GB/s); sign it on ACT ahead of the image-0 signs so the
            # first LDWEIGHTS unblocks as early as possible.
            wstage = wpool.tile([P, KH * KW * NT * C], F32, name="wstage")
            WH = KH * KW * NT * C // 2
            nc.scalar.dma_start(out=wstage[:, :WH], in_=wt_d[:, :WH])
            nc.gpsimd.dma_start(out=wstage[:, WH:], in_=wt_d[:, WH:])
            wsgn = wpool.tile([P, KH * KW * NT * C], FP8, name="wsgn")
            nc.scalar.activation(
                out=wsgn, in_=wstage, func=mybir.ActivationFunctionType.Sign, bias=tiny_t
            )
            # [p, tap, ci_half, co] view for DoubleRow lhsT slices
            wsgn4 = wsgn.rearrange("p (t u co) -> p t u co", u=NT, co=C)

            # ---------------- per-image input pipeline ----------------
            def emit_inputs(n, do_bias=True):
                """DMA x, zero pad borders, sign -> fp8 a-tile, pre-bias x."""
                xlin = []
                # both ci-halves in one fp8 tile: [p, 2, APAD] (DoubleRow rhs)
                at = apool.tile([P, NT * APAD], FP8, name=f"a_{n}", tag="a")
                a2 = at.rearrange("p (u q) -> p u q", u=NT)
                HH = H // 2  # row-halves: lets early-chunk matmuls start
                for u in range(NT):
                    xt = xpool.tile([P, HW], F32, name=f"x_{n}_{u}", tag=f"x{u}")
                    a3 = a2[:, u, 0:NPAD].rearrange("p (h w) -> p h w", w=WP)
                    # zero borders (rows 0,57 and cols 0,57)
                    nc.gpsimd.memset(a3[:, 0, :], 0.0)
                    nc.gpsimd.memset(a3[:, HP - 1, :], 0.0)
                    nc.gpsimd.memset(a3[:, :, 0], 0.0)
                    nc.gpsimd.memset(a3[:, :, WP - 1], 0.0)
                    for h in range(2):
                        rs = slice(h * HH * W_DIM, (h + 1) * HH * W_DIM)
                        nc.sync.dma_start(
                            out=xt[:, rs], in_=x_d[n, u * P : (u + 1) * P, rs]
                        )
                        # sign(x) -> interior; sign(0)=+1 via tiny bias
                        nc.scalar.activation(
                            out=a3[:, 1 + h * HH : 1 + (h + 1) * HH, 1 : W_DIM + 1],
                            in_=xt[:, rs].rearrange("p (h w) -> p h w", w=W_DIM),
                            func=mybir.ActivationFunctionType.Sign,
                            bias=tiny_t,
                        )
                    # xb = x + bias, so PSUM evac is a single (S*scale)+xb op.
                    # Tile orders this in-place write after the sign read (WAR).
                    # Image 0's pre-bias is emitted AFTER the bias computation
                    # below (must not read bias_t before its writer exists).
                    if do_bias:
                        nc.vector.tensor_scalar_add(out=xt, in0=xt, scalar1=bias_t[u])
                    xlin.append(xt)
                return xlin, a2

            pending = {0: emit_inputs(0, do_bias=False)}

            # BN param math (ACT sqrt sits AFTER the image-0 signs + wsgn sign
            # in the ACT queue, so a slow bn DMA cannot delay the first matmul)
            for c in range(NT):
                g_t = bn_t[:, 0, c : c + 1]
                b_t = bn_t[:, 1, c : c + 1]
                m_t = bn_t[:, 2, c : c + 1]
                v_t = bn_t[:, 3, c : c + 1]
                iv, bb = inv_t[c], bias_t[c]
                # inv = gamma / sqrt(var + eps)
                nc.scalar.activation(
                    out=iv, in_=v_t, func=mybir.ActivationFunctionType.Sqrt, bias=eps_t
                )
                nc.vector.reciprocal(out=iv, in_=iv)
                nc.vector.tensor_mul(out=iv, in0=iv, in1=g_t)
                # bias = beta - mean * inv  (pre-added to residual x)
                nc.vector.tensor_mul(out=bb, in0=m_t, in1=iv)
                nc.vector.tensor_sub(out=bb, in0=b_t, in1=bb)

            # image 0's deferred pre-bias (bias_t now has a writer)
            for u in range(NT):
                xt0 = pending[0][0][u]
                nc.vector.tensor_scalar_add(out=xt0, in0=xt0, scalar1=bias_t[u])

            # alpha (mean |W| per co) -> scale = alpha * inv; scalar ring so it
            # queues behind wstage, not in front of x input DMAs
            for c in range(NT):
                wnat = wpool.tile([P, C * KH * KW], F32, name=f"wnat{c}", tag=f"wnat{c}")
                nc.scalar.dma_start(out=wnat, in_=wn_d[c])
                ar = ppool.tile([P, 1], F32, name=f"araw{c}", tag=f"araw{c}")
                nc.vector.reduce_sum(
                    out=ar, in_=wnat, axis=mybir.AxisListType.X, apply_absolute_value=True
                )
                st = scale_t[c]
                nc.vector.tensor_mul(out=st, in0=ar, in1=inv_t[c])
                nc.vector.tensor_scalar_mul(out=st, in0=st, scalar1=1.0 / (C * KH * KW))

            # ---------------- main loop over images ----------------
            PREFETCH = 2
            if n_img > 1:
                pending[1] = emit_inputs(1)
            for n in range(n_img):
                if n + PREFETCH < n_img:
                    pending[n + PREFETCH] = emit_inputs(n + PREFETCH)
                xlin, a2 = pending.pop(n)

                for c in range(NT):
                    for k in range(NCHUNK):
                        w_k = min(CH, NPOS - k * CH)  # 464, last 462
                        ps = pspool.tile([P, CH], F32, name=f"ps_{n}_{c}_{k}", tag="ps")
                        for t in range(KH * KW):
                            ky, kx = divmod(t, KW)
                            off = ky * WP + kx
                            nc.tensor.matmul(
                                ps[:, :w_k],
                                lhsT=wsgn4[:, t, :, c * P : (c + 1) * P],
                                rhs=a2[:, :, k * CH + off : k * CH + off + w_k],
                                start=(t == 0),
                                stop=(t == KH * KW - 1),
                                perf_mode=mybir.MatmulPerfMode.DoubleRow,
                            )
                        # evacuate in one DVE op: xb += S*scale (skip garbage cols)
                        ps3 = ps.rearrange("p (r w) -> p r w", w=WP)
                        xsl3 = xlin[c][:, k * CHV : (k + 1) * CHV].rearrange(
                            "p (r w) -> p r w", w=W_DIM
                        )
                        nc.vector.scalar_tensor_tensor(
                            out=xsl3,
                            in0=ps3[:, :, 0:W_DIM],
                            scalar=scale_t[c],
                            in1=xsl3,
                            op0=mybir.AluOpType.mult,
                            op1=mybir.AluOpType.add,
                        )
                    # outputs ride the scalar ring so they never convoy x
                    # inputs; two halves so the first can fire while the last
                    # chunks still compute
                    for os in (slice(0, 4 * CHV), slice(4 * CHV, HW)):
                        nc.scalar.dma_start(
                            out=out_d[n, c * P : (c + 1) * P, os], in_=xlin[c][:, os]
                        )
    return nc


def kernel(x, W, gamma, beta, running_mean, running_var):
    global LAST_RESULTS
    from concourse.bass_utils import run_bass_kernel_spmd

    assert x.shape == (N_FULL, C, H, W_DIM) and W.shape == (C, C, KH, KW)

    # host-side layout-only prep (no arithmetic): shard batch, permute weights
    xs = np.ascontiguousarray(x.reshape(N_CORES, N_IMG, C, HW))
    # [Cout, Cin, kh, kw] -> [kh,kw][ci_tile][128ci][256co] -> [p, (t u co)]
    w_t = np.ascontiguousarray(
        W.transpose(2, 3, 1, 0)
        .reshape(KH * KW, NT, P, C)
        .transpose(2, 0, 1, 3)
        .reshape(P, KH * KW * NT * C)
    )
    w_n = np.ascontiguousarray(W.reshape(NT, P, C * KH * KW))
    # [4, C] -> [128p, 4param, 2ct] partition-major
    bn = np.ascontiguousarray(
        np.stack([gamma, beta, running_mean, running_var])
        .astype(np.float32)
        .reshape(4, NT, P)
        .transpose(2, 0, 1)
    )

    in_maps = [
        {"x": xs[i], "w_t": w_t, "w_n": w_n, "bn": bn} for i in range(N_CORES)
    ]

    nc = build_nc(N_IMG)
    nc.finalize()
    res = run_bass_kernel_spmd(nc, in_maps, core_ids=list(range(N_CORES)), trace=TRACE)
    LAST_RESULTS = res
    out = np.concatenate([r["out"] for r in res.results], axis=0)
    return np.ascontiguousarray(out.reshape(N_FULL, C, H, W_DIM).astype(np.float32))


# revision 49
# speedup vs baseline: 1.0459x; 1.0075x over previous
"""Trainium2 Bass kernel for nn_BasicBlock_37228776522121 (binary-conv BasicBlock).

Computes: y = conv3x3(binarize(x), sign(W)*alpha) -> BN(inference) -> + x
  where binarize(x) = sign(x) in {-1,+1} (sign(0)=+1),
        alpha[o] = mean(|W[o]|), folded into the BN scale.

Strategy: data-parallel over batch N across 8 NeuronCores (8 images/core).
Per core: conv as 9 shifted matmuls, contraction C=256 done in ONE PE pass
via fp8 DoubleRow (2 K-tiles of 128), sign activations in fp8e4 (exact for
+-1), f32 PSUM accumulation (exact integer sums). BN affine + residual are
fused into a single DVE scalar_tensor_tensor evacuation per PSUM chunk
(bias pre-added to the residual x off the critical path).

Layout trick: activations stored sign-binarized in a width-padded layout
[128, 2, 58*58] so every conv tap (ky,kx) is a contiguous slice at offset
ky*58+kx. Output positions p = h*58+w include 2 garbage columns per row
(w=56,57) skipped when evacuating PSUM (strided 8x56 reads from each
464-wide chunk = 8 padded rows).
"""

import os
import sys

import numpy as np

for _p in ("/opt/trn_rl_repo",):
    if _p not in sys.path and os.path.isdir(_p):
        sys.path.insert(0, _p)

import concourse.bacc as bacc
import concourse.bass as bass
import concourse.tile as tile
from concourse import mybir

# ---- problem constants (hardcoded per contest rules) ----
N_FULL = 64
C = 256  # = Cout
H = 56
W_DIM = 56
KH = KW = 3
BN_EPS = 1e-5
N_CORES = 8
N_IMG = N_FULL // N_CORES  # images per core

P = 128  # partitions
NT = C // P  # channel tiles (2)
HW = H * W_DIM  # 3136
WP = W_DIM + 2  # 58 padded width
HP = H + 2  # 58 padded height
NPAD = HP * WP  # 3364
ROWS_PER_CHUNK = 8
CH = ROWS_PER_CHUNK * WP  # 464 padded positions / chunk
CHV = ROWS_PER_CHUNK * W_DIM  # 448 valid positions / chunk
NCHUNK = H // ROWS_PER_CHUNK  # 7
NPOS = (H - 1) * WP + W_DIM  # 3246 last valid padded index + 1

F32 = mybir.dt.float32
BF16 = mybir.dt.bfloat16
FP8 = mybir.dt.float8e4
APAD = 3376  # NPAD rounded up to 16 for DoubleRow k-tile stride alignment

# set by test.py to collect profile info
TRACE = False
LAST_RESULTS = None


def build_nc(n_img: int = N_IMG):
    nc = bacc.Bacc("TRN2", target_bir_lowering=False)

    x_d = nc.declare_dram_parameter("x", [n_img, C, HW], F32, isOutput=False)
    # weights pre-permuted on host to [128ci, ky*3+kx, ci_tile, 256co] (layout
    # only) so the stage DMA is fully contiguous per partition
    wt_d = nc.declare_dram_parameter("w_t", [P, KH * KW * NT * C], F32, isOutput=False)
    # native-layout weights for alpha: [co_tile, 128co, C*9]
    wn_d = nc.declare_dram_parameter("w_n", [NT, P, C * KH * KW], F32, isOutput=False)
    # gamma/beta/mean/var packed host-side partition-major [128p, 4param, 2ct]
    # so all BN params arrive in ONE contiguous-per-partition DMA (a scattered
    # 4B-element gather here measurably stalls the ACT queue ~20us at startup)
    bn_d = nc.declare_dram_parameter("bn", [P, 4, NT], F32, isOutput=False)
    out_d = nc.declare_dram_parameter("out", [n_img, C, HW], F32, isOutput=True)

    with tile.TileContext(nc) as tc:
        with (
            tc.tile_pool(name="wpool", bufs=1) as wpool,
            tc.tile_pool(name="ppool", bufs=1) as ppool,
            tc.tile_pool(name="xpool", bufs=3) as xpool,
            tc.tile_pool(name="apool", bufs=3) as apool,
            tc.tile_pool(name="pspool", bufs=8, space="PSUM") as pspool,
        ):
            # ---- consts + BN params (tiny DMAs; bias needs no weights) ----
            tiny_t = ppool.tile([P, 1], F32, name="tiny_t")
            nc.vector.memset(tiny_t, 1e-30)
            eps_t = ppool.tile([P, 1], F32, name="eps_t")
            nc.vector.memset(eps_t, BN_EPS)
            # warm the ACT function tables now so their table-data DMAs queue
            # ahead of the bulk x/W transfers (otherwise the first real Sign
            # stalls ~15us mid-startup waiting for its table)
            warm_t = ppool.tile([P, 1], F32, name="warm_t")
            nc.scalar.activation(
                out=warm_t, in_=tiny_t, func=mybir.ActivationFunctionType.Sign, bias=tiny_t
            )
            nc.scalar.activation(
                out=warm_t, in_=warm_t, func=mybir.ActivationFunctionType.Sqrt, bias=eps_t
            )

            # one contiguous DMA for all BN params; computes are emitted LATER
            # (after the image-0 signs) so they can't convoy the ACT queue.
            # Tiles are allocated now so emit_inputs can reference bias_t.
            bn_t = ppool.tile([P, 4, NT], F32, name="bn_t")
            nc.sync.dma_start(out=bn_t, in_=bn_d[:])
            inv_t = [ppool.tile([P, 1], F32, name=f"inv{c}", tag=f"inv{c}") for c in range(NT)]
            bias_t = [ppool.tile([P, 1], F32, name=f"bb{c}", tag=f"bb{c}") for c in range(NT)]
            scale_t = [ppool.tile([P, 1], F32, name=f"s{c}", tag=f"s{c}") for c in range(NT)]

            # ---- weight stage split across the scalar + gpsimd rings so the
            # two halves transfer concurrently (a single ~2.3MB DMA measures
            # only ~180GB/s); sign it on ACT ahead of the image-0 signs so the
            # first LDWEIGHTS unblocks as early as possible.
            wstage = wpool.tile([P, KH * KW * NT * C], F32, name="wstage")
            WH = KH * KW * NT * C // 2
            nc.scalar.dma_start(out=wstage[:, :WH], in_=wt_d[:, :WH])
            nc.gpsimd.dma_start(out=wstage[:, WH:], in_=wt_d[:, WH:])
            wsgn = wpool.tile([P, KH * KW * NT * C], FP8, name="wsgn")
            for ws in (slice(0, WH), slice(WH, KH * KW * NT * C)):
                nc.scalar.activation(
                    out=wsgn[:, ws],
                    in_=wstage[:, ws],
                    func=mybir.ActivationFunctionType.Sign,
                    bias=tiny_t,
                )
            # [p, tap, ci_half, co] view for DoubleRow lhsT slices
            wsgn4 = wsgn.rearrange("p (t u co) -> p t u co", u=NT, co=C)

            # ---------------- per-image input pipeline ----------------
            img_dmas = {}  # n -> (first_dma, last_dma) for priority chaining

            def emit_inputs(n, do_bias=True):
                """DMA x, zero pad borders, sign -> fp8 a-tile, pre-bias x."""
                xlin = []
                # both ci-halves in one fp8 tile: [p, 2, APAD] (DoubleRow rhs)
                at = apool.tile([P, NT * APAD], FP8, name=f"a_{n}", tag="a")
                a2 = at.rearrange("p (u q) -> p u q", u=NT)
                HH = H // 2  # row-halves: lets early-chunk matmuls start
                for u in range(NT):
                    xt = xpool.tile([P, HW], F32, name=f"x_{n}_{u}", tag=f"x{u}")
                    a3 = a2[:, u, 0:NPAD].rearrange("p (h w) -> p h w", w=WP)
                    # zero borders (rows 0,57 and cols 0,57)
                    nc.gpsimd.memset(a3[:, 0, :], 0.0)
                    nc.gpsimd.memset(a3[:, HP - 1, :], 0.0)
                    nc.gpsimd.memset(a3[:, :, 0], 0.0)
                    nc.gpsimd.memset(a3[:, :, WP - 1], 0.0)
                    for h in range(2):
                        rs = slice(h * HH * W_DIM, (h + 1) * HH * W_DIM)
                        dma = nc.sync.dma_start(
                            out=xt[:, rs], in_=x_d[n, u * P : (u + 1) * P, rs]
                        )
                        if n not in img_dmas:
                            img_dmas[n] = (dma, dma)
                        else:
                            img_dmas[n] = (img_dmas[n][0], dma)
                        # sign(x) -> interior; sign(0)=+1 via tiny bias
                        nc.scalar.activation(
                            out=a3[:, 1 + h * HH : 1 + (h + 1) * HH, 1 : W_DIM + 1],
                            in_=xt[:, rs].rearrange("p (h w) -> p h w", w=W_DIM),
                            func=mybir.ActivationFunctionType.Sign,
                            bias=tiny_t,
                        )
                    # xb = x + bias, so PSUM evac is a single (S*scale)+xb op.
                    # Tile orders this in-place write after the sign read (WAR).
                    # Image 0's pre-bias is emitted AFTER the bias computation
                    # below (must not read bias_t before its writer exists).
                    if do_bias:
                        nc.vector.tensor_scalar_add(out=xt, in0=xt, scalar1=bias_t[u])
                    xlin.append(xt)
                # keep the startup fabric clear for image 0: images 1 and 2
                # only start their x transfers once the previous image landed
                if n in (1, 2) and (n - 1) in img_dmas:
                    tile.add_dep_helper(
                        img_dmas[n][0].ins,
                        img_dmas[n - 1][1].ins,
                        sync=True,
                        reason="startup x DMA priority",
                    )
                return xlin, a2

            pending = {0: emit_inputs(0, do_bias=False)}

            # BN param math (ACT sqrt sits AFTER the image-0 signs + wsgn sign
            # in the ACT queue, so a slow bn DMA cannot delay the first matmul)
            for c in range(NT):
                g_t = bn_t[:, 0, c : c + 1]
                b_t = bn_t[:, 1, c : c + 1]
                m_t = bn_t[:, 2, c : c + 1]
                v_t = bn_t[:, 3, c : c + 1]
                iv, bb = inv_t[c], bias_t[c]
                # inv = gamma / sqrt(var + eps)
                nc.scalar.activation(
                    out=iv, in_=v_t, func=mybir.ActivationFunctionType.Sqrt, bias=eps_t
                )
                nc.vector.reciprocal(out=iv, in_=iv)
                nc.vector.tensor_mul(out=iv, in0=iv, in1=g_t)
                # bias = beta - mean * inv  (pre-added to residual x)
                nc.vector.tensor_mul(out=bb, in0=m_t, in1=iv)
                nc.vector.tensor_sub(out=bb, in0=b_t, in1=bb)

            # image 0's deferred pre-bias (bias_t now has a writer)
            for u in range(NT):
                xt0 = pending[0][0][u]
                nc.vector.tensor_scalar_add(out=xt0, in0=xt0, scalar1=bias_t[u])

            # alpha (mean |W| per co) -> scale = alpha * inv; scalar ring so it
            # queues behind wstage, not in front of x input DMAs
            for c in range(NT):
                wnat = wpool.tile([P, C * KH * KW], F32, name=f"wnat{c}", tag=f"wnat{c}")
                nc.scalar.dma_start(out=wnat, in_=wn_d[c])
                ar = ppool.tile([P, 1], F32, name=f"araw{c}", tag=f"araw{c}")
                nc.vector.reduce_sum(
                    out=ar, in_=wnat, axis=mybir.AxisListType.X, apply_absolute_value=True
                )
                st = scale_t[c]
                nc.vector.tensor_mul(out=st, in0=ar, in1=inv_t[c])
                nc.vector.tensor_scalar_mul(out=st, in0=st, scalar1=1.0 / (C * KH * KW))

            # ---------------- main loop over images ----------------
            PREFETCH = 2
            if n_img > 1:
                pending[1] = emit_inputs(1)
            for n in range(n_img):
                if n + PREFETCH < n_img:
                    pending[n + PREFETCH] = emit_inputs(n + PREFETCH)
                xlin, a2 = pending.pop(n)

                for c in range(NT):
                    for k in range(NCHUNK):
                        w_k = min(CH, NPOS - k * CH)  # 464, last 462
                        ps = pspool.tile([P, CH], F32, name=f"ps_{n}_{c}_{k}", tag="ps")
                        for t in range(KH * KW):
                            ky, kx = divmod(t, KW)
                            off = ky * WP + kx
                            nc.tensor.matmul(
                                ps[:, :w_k],
                                lhsT=wsgn4[:, t, :, c * P : (c + 1) * P],
                                rhs=a2[:, :, k * CH + off : k * CH + off + w_k],
                                start=(t == 0),
                                stop=(t == KH * KW - 1),
                                perf_mode=mybir.MatmulPerfMode.DoubleRow,
                            )
                        # evacuate in one DVE op: xb += S*scale (skip garbage cols)
                        ps3 = ps.rearrange("p (r w) -> p r w", w=WP)
                        xsl3 = xlin[c][:, k * CHV : (k + 1) * CHV].rearrange(
                            "p (r w) -> p r w", w=W_DIM
                        )
                        nc.vector.scalar_tensor_tensor(
                            out=xsl3,
                            in0=ps3[:, :, 0:W_DIM],
                            scalar=scale_t[c],
                            in1=xsl3,
                            op0=mybir.AluOpType.mult,
                            op1=mybir.AluOpType.add,
                        )
                    # outputs ride the scalar ring so they never convoy x
                    # inputs; two halves so the first can fire while the last
                    # chunks still compute
                    for os in (slice(0, 4 * CHV), slice(4 * CHV, HW)):
                        nc.scalar.dma_start(
                            out=out_d[n, c * P : (c + 1) * P, os], in_=xlin[c][:, os]
                        )
    return nc


def kernel(x, W, gamma, beta, running_mean, running_var):
    global LAST_RESULTS
    from concourse.bass_utils import run_bass_kernel_spmd

    assert x.shape == (N_FULL, C, H, W_DIM) and W.shape == (C, C, KH, KW)

    # host-side layout-only prep (no arithmetic): shard batch, permute weights
    xs = np.ascontiguousarray(x.reshape(N_CORES, N_IMG, C, HW))
    # [Cout, Cin, kh, kw] -> [kh,kw][ci_tile][128ci][256co] -> [p, (t u co)]
    w_t = np.ascontiguousarray(
        W.transpose(2, 3, 1, 0)
        .reshape(KH * KW, NT, P, C)
        .transpose(2, 0, 1, 3)
        .reshape(P, KH * KW * NT * C)
    )
    w_n = np.ascontiguousarray(W.reshape(NT, P, C * KH * KW))
    # [4, C] -> [128p, 4param, 2ct] partition-major
    bn = np.ascontiguousarray(
        np.stack([gamma, beta, running_mean, running_var])
        .astype(np.float32)
        .reshape(4, NT, P)
        .transpose(2, 0, 1)
    )

    in_maps = [
        {"x": xs[i], "w_t": w_t, "w_n": w_n, "bn": bn} for i in range(N_CORES)
    ]

    nc = build_nc(N_IMG)
    nc.finalize()
    res = run_bass_kernel_spmd(nc, in_maps, core_ids=list(range(N_CORES)), trace=TRACE)
    LAST_RESULTS = res
    out = np.concatenate([r["out"] for r in res.results], axis=0)
    return np.ascontiguousarray(out.reshape(N_FULL, C, H, W_DIM).astype(np.float32))
